# revision 18
# baseline (speedup 1.0000x reference)
"""Trainium2 Bass kernel for nn_BasisAffinityGAT (B=8, N=512, D=R=128, K=8).

Math (matches reference.py):
    fused = concat(desc, nve) @ W_fuse + b_fuse                 [B,N,D]
    q = fused @ W_q[k];  kk = fused @ W_k[k]                    per basis
    e_q[b,k,n] = lrelu(q).a_q[k];  e_k likewise
    logits = e_q[:,:,:,None] + e_k[:,:,None,:], symmetrized
    alpha  = softmax(logits, -1); ema update; bias_log = log(clip(ema'))

Exact algebra used:
  * sym-logits[i,j] = 0.5*(s_i + s_j) with s = e_q + e_k, so the row
    softmax collapses: alpha[b,k,i,j] = softmax_j(0.5*s[b,k,:])[j],
    independent of i.
  * lrelu(x) = 0.6*x + 0.4*|x| (slope 0.2), so
    0.5*s[b,k,n] = x_cat[b,n,:] @ wl2[k]
                   + c0[k] + 0.2*(a_q[k] . |q_T|) + 0.2*(a_k[k] . |k_T|)
  * the fusion layer is folded into the per-basis projections on host:
        q = x_cat @ (W_fuse @ W_q[k]) + b_fuse @ W_q[k] =: x_cat@Wxq + bq
    (same for k), wl2 = W_fuse @ wlin, c0 = b_fuse . wlin with
    wlin = 0.3*(W_q[k] @ a_q[k] + W_k[k] @ a_k[k]).  The bq/bk adds ride
    the ACT Abs activations (bias_ptr), c0 rides the Exp bias.
  * bias_log content is batch-independent ([K,N,N] broadcast over B).

Sharding (8 cores, SPMD, zero cross-core communication): core m owns
basis k=m for ALL batches.

v8 performance structure (the kernel is output-write-bound: 16 MiB of
DRAM writes per core vs ~2.4 MB of reads; a single HWDGE ring sustains
~380 GB/s only while backlogged, and per-op trigger/receipt overheads
plus compute-cadence gaps starve it when the board clock is throttled):
  * all bf16 weights ship as ONE packed [128,516] DRAM tensor and the
    f32 biases as one [128,3] tensor — a HWDGE ring serializes ~1.3us
    of completion latency per DMA op, so 9 small loads would push xb1
    (queued behind them) out by ~10us.
  * every PSUM accumulation group is dtype-pure (all-bf16 or all-f32r):
    mixing bf16 and f32r matmuls in one open group corrupts the PSUM
    contents on HW (pairwise even/odd column garbage; CoreSim does not
    model it).
  * folding the fusion layer removes the fuse matmuls + a DVE hop from
    every batch's dependency chain.
  * a dummy compute chain on zeroed tiles runs right after the preamble
    so every engine is clock-warm when xb0 lands (cold engines take
    ~1.5-2.5us to wake on a semaphore; warm ones ~40ns).
  * output writes alternate across BOTH HWDGE rings (even alpha batches
    + bias chunks 0/2 on sync, odd ones on scalar): consecutive writes
    overlap their per-op trigger/receipt latencies, which keeps the
    write stream at line rate even in throttled-clock board states
    (~10us faster there than a single-ring write stream).
  * xb0 and xb1 are the first ops on the sync HWDGE ring and the weight
    packs lead the scalar ring, so nothing gates batch 0/1 compute.
    xb2..7 + ema prefetch on the gpsimd SWDGE ring.
  * deep bufs decouple compute from the write backlog.
  * pbar (batch-mean of p) is accumulated on a [1,N] lane and broadcast
    to 128 partitions once at the end.
"""

import sys

import numpy as np

if "/opt/trn_rl_repo" not in sys.path:
    sys.path.insert(0, "/opt/trn_rl_repo")

from contextlib import ExitStack

import ml_dtypes

import concourse.bass as bass
import concourse.tile as tile
from concourse import bacc, mybir
from concourse.bass_utils import run_bass_kernel_spmd

B, N, D, K = 8, 512, 128, 8
R = D
MOM = 0.99
EPS = 1e-6
N_CORES = 8
F32 = mybir.dt.float32
F32R = mybir.dt.float32r
BF16 = mybir.dt.bfloat16
AF = mybir.ActivationFunctionType
ALU = mybir.AluOpType
PBAR_C = 0.01 / B / MOM  # (1-MOM)/B scaled so Ln(scale=MOM) folds MOM back
WCOLS = 4 * R + 4  # packed bf16 weight columns


def build():
    """Build the SPMD per-core Bass program (identical on all 8 cores)."""
    nc = bacc.Bacc("TRN2", target_bir_lowering=False, debug=False,
                   num_devices=N_CORES)

    # ---- per-core external tensors -------------------------------------
    # xTall[b,h,d,n]: h=0 desc[b].T, h=1 nve[b].T  (same array on all cores)
    xTall = nc.dram_tensor("xTall", [B, 2, D, N], BF16, kind="ExternalInput")
    # wpack[d] = [Wxq_h0[d] | Wxq_h1[d] | Wxk_h0[d] | Wxk_h1[d] |
    #             wl2_h0[d], wl2_h1[d], 0.2aq[d], 0.2ak[d]]
    wpack = nc.dram_tensor("wpack", [D, WCOLS], BF16, kind="ExternalInput")
    bpack = nc.dram_tensor("bpack", [D, 3], F32, kind="ExternalInput")
    ema = nc.dram_tensor("ema", [N, N], BF16, kind="ExternalInput")  # [m]
    alpha = nc.dram_tensor("alpha", [B, N, N], F32, kind="ExternalOutput")
    biaso = nc.dram_tensor("bias", [B, N, N], F32, kind="ExternalOutput")

    with ExitStack() as ctx:
        tc = ctx.enter_context(tile.TileContext(nc))
        const = ctx.enter_context(tc.tile_pool(name="const", bufs=1))
        work = ctx.enter_context(tc.tile_pool(name="work", bufs=2))
        absp = ctx.enter_context(tc.tile_pool(name="absp", bufs=4))
        psum = ctx.enter_context(tc.tile_pool(name="psum", bufs=1, space="PSUM"))

        wpack_sb = const.tile([D, WCOLS], BF16)
        bpack_sb = const.tile([D, 3], F32)
        ones1_sb = const.tile([1, D], F32)
        onesc_sb = const.tile([1, D], F32R)
        pbar_sb = const.tile([1, N], F32R)
        ema_sb = const.tile([128, 4 * N], F32)

        wxq0 = wpack_sb[:, 0 * R:1 * R]
        wxq1 = wpack_sb[:, 1 * R:2 * R]
        wxk0 = wpack_sb[:, 2 * R:3 * R]
        wxk1 = wpack_sb[:, 3 * R:4 * R]
        wl20 = wpack_sb[:, 4 * R + 0:4 * R + 1]
        wl21 = wpack_sb[:, 4 * R + 1:4 * R + 2]
        aq_l = wpack_sb[:, 4 * R + 2:4 * R + 3]
        ak_l = wpack_sb[:, 4 * R + 3:4 * R + 4]
        bq_l = bpack_sb[:, 0:1]
        bk_l = bpack_sb[:, 1:2]
        c0_l = bpack_sb[0:1, 2:3]

        xb_tiles = [absp.tile([D, 2 * N], BF16, tag="xb", bufs=B,
                              name=f"xb{b}") for b in range(B)]

        # xb0 first on the sync ring (alpha writes follow it there);
        # packed weights + xb1 on the scalar ring (bias writes follow);
        # the SWDGE ring prefetches the rest (its first packet is ~5us
        # late, fine for xb2+).
        nc.sync.dma_start(
            xb_tiles[0][:].rearrange("d (h n) -> d h n", h=2),
            xTall[0].rearrange("h d n -> d h n"))
        nc.sync.dma_start(
            xb_tiles[1][:].rearrange("d (h n) -> d h n", h=2),
            xTall[1].rearrange("h d n -> d h n"))
        nc.scalar.dma_start(wpack_sb[:], wpack[:])
        nc.scalar.dma_start(bpack_sb[:], bpack[:])
        for b in range(2, B):
            nc.gpsimd.dma_start(
                xb_tiles[b][:].rearrange("d (h n) -> d h n", h=2),
                xTall[b].rearrange("h d n -> d h n"))
        # ema: bf16 in DRAM, cast to f32 during the SWDGE transfer
        nc.gpsimd.dma_start(
            ema_sb[:].rearrange("p (c n) -> p c n", c=4),
            ema.ap().rearrange("(c p) n -> p c n", p=128))
        nc.vector.memset(ones1_sb[:], 1.0)
        nc.vector.tensor_copy(onesc_sb[:], ones1_sb[:])  # f32r copy of ones

        # ---- engine warm-up: a full dummy chain on zeroed tiles --------
        # (no DRAM deps; keeps PE/ACT/DVE clocks up so batch 0's real
        # chain sees ~40ns semaphore wakeups instead of ~2us ones)
        dummy_x = const.tile([D, N], BF16)
        nc.vector.memset(dummy_x[:], 0.0)
        for w in range(2):
            psum_d = psum.tile([D, N], F32, tag="mm", bufs=4, name=f"psd{w}")
            nc.tensor.matmul(psum_d[:], dummy_x[:, 0:D], dummy_x[:],
                             start=True, stop=True)
            absd = absp.tile([D, N], BF16, tag="abs", bufs=4, name=f"absd{w}")
            nc.scalar.activation(absd[:], psum_d[:], AF.Abs)
            dume = work.tile([1, 1], F32, tag="se", bufs=6, name=f"dume{w}")
            dexp = work.tile([1, N], F32R, tag="ex", bufs=6, name=f"dexp{w}")
            nc.scalar.activation(dexp[:], psum_d[0:1, :], AF.Exp,
                                 scale=1.0, accum_out=dume[:])
            drs = work.tile([1, 1], F32, tag="rs", bufs=6, name=f"drs{w}")
            nc.vector.reciprocal(drs[:], dume[:])
            drr = work.tile([1, D], F32R, tag="rr", bufs=6, name=f"drr{w}")
            nc.vector.tensor_scalar_mul(drr[:], ones1_sb[:], drs[:])
            dcp = work.tile([128, N], F32, tag="repsb", bufs=B + 2,
                            name=f"dcp{w}")
            nc.vector.tensor_copy(dcp[:], psum_d[:])

        for b in range(B):
            xb = xb_tiles[b]
            psum_q = psum.tile([D, N], F32, tag="mm", bufs=4)
            nc.tensor.matmul(psum_q[:], wxq0, xb[:, 0:N],
                             start=True, stop=False)
            nc.tensor.matmul(psum_q[:], wxq1, xb[:, N:2 * N],
                             start=False, stop=True)
            absq = absp.tile([D, N], BF16, tag="abs", bufs=4)
            nc.scalar.activation(absq[:], psum_q[:], AF.Abs, bias=bq_l)
            psum_s = psum.tile([1, N], F32, tag="ps", bufs=2)
            nc.tensor.matmul(psum_s[:], wl20, xb[:, 0:N],
                             start=True, stop=False)
            nc.tensor.matmul(psum_s[:], wl21, xb[:, N:2 * N],
                             start=False, stop=False)
            psum_k = psum.tile([D, N], F32, tag="mm", bufs=4)
            nc.tensor.matmul(psum_k[:], wxk0, xb[:, 0:N],
                             start=True, stop=False)
            nc.tensor.matmul(psum_k[:], wxk1, xb[:, N:2 * N],
                             start=False, stop=True)
            absk = absp.tile([D, N], BF16, tag="abs", bufs=4)
            nc.scalar.activation(absk[:], psum_k[:], AF.Abs, bias=bk_l)
            nc.tensor.matmul(psum_s[:], aq_l, absq[:],
                             start=False, stop=False)
            nc.tensor.matmul(psum_s[:], ak_l, absk[:],
                             start=False, stop=True)

            # ---- softmax over free dim (no max-shift: |s| is O(1), exp
            # is safe in fp32 and softmax is shift-invariant) -------------
            expv = work.tile([1, N], F32R, tag="ex", bufs=6)
            sume = work.tile([1, 1], F32, tag="se", bufs=6)
            nc.scalar.activation(expv[:], psum_s[:], AF.Exp,
                                 scale=1.0, bias=c0_l, accum_out=sume[:])
            rsum = work.tile([1, 1], F32, tag="rs", bufs=6)
            nc.vector.reciprocal(rsum[:], sume[:])

            # pbar += PBAR_C * p  on a single [1,N] lane
            rsum_c = work.tile([1, 1], F32, tag="rc", bufs=6)
            nc.vector.tensor_scalar_mul(rsum_c[:], rsum[:], PBAR_C)
            if b == 0:
                nc.vector.tensor_scalar(pbar_sb[:], expv[:], rsum_c[:], None,
                                        op0=ALU.mult)
            else:
                nc.vector.scalar_tensor_tensor(
                    pbar_sb[:], expv[:], rsum_c[:], pbar_sb[:],
                    op0=ALU.mult, op1=ALU.add)

            # ---- alpha[b, i, :] = p_b for all i ------------------------
            # broadcast via PE: lhsT = rsum replicated (fp32r) so the
            # matmul computes rsum*expv = p on all 128 partitions.
            rsum_rep = work.tile([1, D], F32R, tag="rr", bufs=6)
            nc.vector.tensor_scalar_mul(rsum_rep[:], ones1_sb[:], rsum[:])
            psum_rep = psum.tile([128, N], F32, tag="rep", bufs=2)
            nc.tensor.matmul(psum_rep[:], rsum_rep[:], expv[:],
                             start=True, stop=True)
            rep_t = work.tile([128, N], F32, tag="repsb", bufs=B + 2)
            nc.vector.tensor_copy(rep_t[:], psum_rep[:])
            src = rep_t[:].rearrange(
                "p (o n) -> p o n", o=1).broadcast_to([128, 4, N])
            dst = alpha[b].rearrange("(p i) j -> p i j", p=128)
            if b % 2 == 0:
                nc.sync.dma_start(dst, src)
            else:
                nc.scalar.dma_start(dst, src)

        # ---- bias_log: broadcast pbar once, then 4 row-chunks ----------
        psum_pb = psum.tile([128, N], F32, tag="rep", bufs=2)
        nc.tensor.matmul(psum_pb[:], onesc_sb[:], pbar_sb[:],
                         start=True, stop=True)
        for c in range(4):
            u = work.tile([128, N], F32, tag="u", bufs=4)
            nc.vector.tensor_add(u[:], ema_sb[:, bass.ts(c, N)], psum_pb[:])
            v = work.tile([128, N], F32, tag="v", bufs=4)
            nc.vector.tensor_scalar_max(v[:], u[:], EPS / MOM)
            bias_t = work.tile([128, N], F32, tag="biassb", bufs=4)
            nc.scalar.activation(bias_t[:], v[:], AF.Ln, scale=MOM)
            src = bias_t[:].rearrange(
                "p (o n) -> p o n", o=1).broadcast_to([128, B, N])
            dst = biaso.ap().rearrange("b (c p) j -> c p b j", c=4)[c]
            if c % 2 == 0:
                nc.sync.dma_start(dst, src)
            else:
                nc.scalar.dma_start(dst, src)

    nc.compile()
    return nc


_NC_CACHE = None


def _get_nc():
    global _NC_CACHE
    if _NC_CACHE is None:
        _NC_CACHE = build()
    return _NC_CACHE


def make_in_maps(desc_embeddings, name_value_embeddings, W_fuse, b_fuse,
                 W_q, W_k, a, alpha_ema):
    """Host-side sharding / weight prep -> per-core input dicts."""
    bf16 = ml_dtypes.bfloat16
    desc = np.asarray(desc_embeddings, np.float32)
    nve = np.asarray(name_value_embeddings, np.float32)
    W_fuse = np.asarray(W_fuse, np.float32)
    b_fuse = np.asarray(b_fuse, np.float32)
    W_q = np.asarray(W_q, np.float32)
    W_k = np.asarray(W_k, np.float32)
    a = np.asarray(a, np.float32)
    alpha_ema = np.asarray(alpha_ema, np.float32)

    a_q = a[:, :R, 0]                      # [K,R]
    a_k = a[:, R:, 0]                      # [K,R]
    wlin = 0.3 * (np.einsum("kdr,kr->kd", W_q, a_q)
                  + np.einsum("kdr,kr->kd", W_k, a_k))  # [K,D]

    # fold the fusion layer into the per-basis projections
    Wxq = np.einsum("de,ker->kdr", W_fuse, W_q)   # [K,2D,R]
    Wxk = np.einsum("de,ker->kdr", W_fuse, W_k)
    bqv = np.einsum("e,ker->kr", b_fuse, W_q)     # [K,R]
    bkv = np.einsum("e,ker->kr", b_fuse, W_k)
    wl2v = np.einsum("de,ke->kd", W_fuse, wlin)   # [K,2D]
    c0v = wlin @ b_fuse                            # [K]

    # xTall[b] = [desc[b].T, nve[b].T] — shared across cores, bf16
    xTall = np.ascontiguousarray(
        np.stack([np.stack([desc[b].T, nve[b].T], axis=0)
                  for b in range(B)], axis=0)).astype(bf16)

    shared = dict(xTall=xTall)
    in_maps = []
    for m in range(N_CORES):
        wp = np.empty((D, WCOLS), np.float32)
        wp[:, 0 * R:1 * R] = Wxq[m][:D]
        wp[:, 1 * R:2 * R] = Wxq[m][D:]
        wp[:, 2 * R:3 * R] = Wxk[m][:D]
        wp[:, 3 * R:4 * R] = Wxk[m][D:]
        wp[:, 4 * R + 0] = wl2v[m][:D]
        wp[:, 4 * R + 1] = wl2v[m][D:]
        wp[:, 4 * R + 2] = 0.2 * a_q[m]
        wp[:, 4 * R + 3] = 0.2 * a_k[m]
        bp = np.empty((D, 3), np.float32)
        bp[:, 0] = bqv[m]
        bp[:, 1] = bkv[m]
        bp[:, 2] = c0v[m]
        in_maps.append(dict(
            shared,
            wpack=np.ascontiguousarray(wp).astype(bf16),
            bpack=np.ascontiguousarray(bp),
            ema=np.ascontiguousarray(alpha_ema[m]).astype(bf16)))
    return in_maps


def gather(results):
    alpha_full = np.stack([r["alpha"] for r in results], axis=1)
    bias_full = np.stack([r["bias"] for r in results], axis=1)
    return bias_full, alpha_full


def kernel(**inputs):
    nc = _get_nc()
    in_maps = make_in_maps(**inputs)
    res = run_bass_kernel_spmd(nc, in_maps, list(range(N_CORES)))
    return gather(res.results)


# revision 19
# speedup vs baseline: 1.0180x; 1.0180x over previous
"""Trainium2 Bass kernel for nn_BasisAffinityGAT (B=8, N=512, D=R=128, K=8).

Math (matches reference.py):
    fused = concat(desc, nve) @ W_fuse + b_fuse                 [B,N,D]
    q = fused @ W_q[k];  kk = fused @ W_k[k]                    per basis
    e_q[b,k,n] = lrelu(q).a_q[k];  e_k likewise
    logits = e_q[:,:,:,None] + e_k[:,:,None,:], symmetrized
    alpha  = softmax(logits, -1); ema update; bias_log = log(clip(ema'))

Exact algebra used:
  * sym-logits[i,j] = 0.5*(s_i + s_j) with s = e_q + e_k, so the row
    softmax collapses: alpha[b,k,i,j] = softmax_j(0.5*s[b,k,:])[j],
    independent of i.
  * lrelu(x) = 0.6*x + 0.4*|x| (slope 0.2), so
    0.5*s[b,k,n] = x_cat[b,n,:] @ wl2[k]
                   + c0[k] + 0.2*(a_q[k] . |q_T|) + 0.2*(a_k[k] . |k_T|)
  * the fusion layer is folded into the per-basis projections on host:
        q = x_cat @ (W_fuse @ W_q[k]) + b_fuse @ W_q[k] =: x_cat@Wxq + bq
    (same for k), wl2 = W_fuse @ wlin, c0 = b_fuse . wlin with
    wlin = 0.3*(W_q[k] @ a_q[k] + W_k[k] @ a_k[k]).  The bq/bk adds ride
    the ACT Abs activations (bias_ptr), c0 rides the Exp bias.
  * bias_log content is batch-independent ([K,N,N] broadcast over B).

Sharding (8 cores, SPMD, zero cross-core communication): core m owns
basis k=m for ALL batches.

v8 performance structure (the kernel is output-write-bound: 16 MiB of
DRAM writes per core vs ~2.4 MB of reads; a single HWDGE ring sustains
~380 GB/s only while backlogged, and per-op trigger/receipt overheads
plus compute-cadence gaps starve it when the board clock is throttled):
  * all bf16 weights ship as ONE packed [128,516] DRAM tensor and the
    f32 biases as one [128,3] tensor — a HWDGE ring serializes ~1.3us
    of completion latency per DMA op, so 9 small loads would push xb1
    (queued behind them) out by ~10us.
  * every PSUM accumulation group is dtype-pure (all-bf16 or all-f32r):
    mixing bf16 and f32r matmuls in one open group corrupts the PSUM
    contents on HW (pairwise even/odd column garbage; CoreSim does not
    model it).
  * folding the fusion layer removes the fuse matmuls + a DVE hop from
    every batch's dependency chain.
  * a dummy compute chain on zeroed tiles runs right after the preamble
    so every engine is clock-warm when xb0 lands (cold engines take
    ~1.5-2.5us to wake on a semaphore; warm ones ~40ns).
  * output writes alternate across BOTH HWDGE rings (even alpha batches
    + bias chunks 0/2 on sync, odd ones on scalar): consecutive writes
    overlap their per-op trigger/receipt latencies, which keeps the
    write stream at line rate even in throttled-clock board states
    (~10us faster there than a single-ring write stream).
  * xb0 and xb1 are the first ops on the sync HWDGE ring and the weight
    packs lead the scalar ring, so nothing gates batch 0/1 compute.
    xb2..7 + ema prefetch on the gpsimd SWDGE ring.
  * deep bufs decouple compute from the write backlog.
  * pbar (batch-mean of p) is accumulated on a [1,N] lane and broadcast
    to 128 partitions once at the end.
"""

import sys

import numpy as np

if "/opt/trn_rl_repo" not in sys.path:
    sys.path.insert(0, "/opt/trn_rl_repo")

from contextlib import ExitStack

import ml_dtypes

import concourse.bass as bass
import concourse.tile as tile
from concourse import bacc, mybir
from concourse.bass_utils import run_bass_kernel_spmd

B, N, D, K = 8, 512, 128, 8
R = D
MOM = 0.99
EPS = 1e-6
N_CORES = 8
F32 = mybir.dt.float32
F32R = mybir.dt.float32r
BF16 = mybir.dt.bfloat16
AF = mybir.ActivationFunctionType
ALU = mybir.AluOpType
PBAR_C = 0.01 / B / MOM  # (1-MOM)/B scaled so Ln(scale=MOM) folds MOM back
WCOLS = 4 * R + 4  # packed bf16 weight columns


def build():
    """Build the SPMD per-core Bass program (identical on all 8 cores)."""
    nc = bacc.Bacc("TRN2", target_bir_lowering=False, debug=False,
                   num_devices=N_CORES)

    # ---- per-core external tensors -------------------------------------
    # xTall[b,h,d,n]: h=0 desc[b].T, h=1 nve[b].T  (same array on all cores)
    xTall = nc.dram_tensor("xTall", [B, 2, D, N], BF16, kind="ExternalInput")
    # wpack[d] = [Wxq_h0[d] | Wxq_h1[d] | Wxk_h0[d] | Wxk_h1[d] |
    #             wl2_h0[d], wl2_h1[d], 0.2aq[d], 0.2ak[d]]
    wpack = nc.dram_tensor("wpack", [D, WCOLS], BF16, kind="ExternalInput")
    bpack = nc.dram_tensor("bpack", [D, 3], F32, kind="ExternalInput")
    ema = nc.dram_tensor("ema", [N, N], BF16, kind="ExternalInput")  # [m]
    alpha = nc.dram_tensor("alpha", [B, N, N], F32, kind="ExternalOutput")
    biaso = nc.dram_tensor("bias", [B, N, N], F32, kind="ExternalOutput")

    with ExitStack() as ctx:
        tc = ctx.enter_context(tile.TileContext(nc))
        const = ctx.enter_context(tc.tile_pool(name="const", bufs=1))
        work = ctx.enter_context(tc.tile_pool(name="work", bufs=2))
        absp = ctx.enter_context(tc.tile_pool(name="absp", bufs=4))
        psum = ctx.enter_context(tc.tile_pool(name="psum", bufs=1, space="PSUM"))

        wpack_sb = const.tile([D, WCOLS], BF16)
        bpack_sb = const.tile([D, 3], F32)
        ones1_sb = const.tile([1, D], F32)
        onesc_sb = const.tile([1, D], F32R)
        pbar_sb = const.tile([1, N], F32R)
        ema_sb = const.tile([128, 4 * N], F32)

        wxq0 = wpack_sb[:, 0 * R:1 * R]
        wxq1 = wpack_sb[:, 1 * R:2 * R]
        wxk0 = wpack_sb[:, 2 * R:3 * R]
        wxk1 = wpack_sb[:, 3 * R:4 * R]
        wl20 = wpack_sb[:, 4 * R + 0:4 * R + 1]
        wl21 = wpack_sb[:, 4 * R + 1:4 * R + 2]
        aq_l = wpack_sb[:, 4 * R + 2:4 * R + 3]
        ak_l = wpack_sb[:, 4 * R + 3:4 * R + 4]
        bq_l = bpack_sb[:, 0:1]
        bk_l = bpack_sb[:, 1:2]
        c0_l = bpack_sb[0:1, 2:3]

        xb_tiles = [absp.tile([D, 2 * N], BF16, tag="xb", bufs=B,
                              name=f"xb{b}") for b in range(B)]

        # xb0 first on the sync ring (alpha writes follow it there);
        # packed weights + xb1 on the scalar ring (bias writes follow);
        # the SWDGE ring prefetches the rest (its first packet is ~5us
        # late, fine for xb2+).
        nc.sync.dma_start(
            xb_tiles[0][:].rearrange("d (h n) -> d h n", h=2),
            xTall[0].rearrange("h d n -> d h n"))
        nc.sync.dma_start(
            xb_tiles[1][:].rearrange("d (h n) -> d h n", h=2),
            xTall[1].rearrange("h d n -> d h n"))
        nc.scalar.dma_start(wpack_sb[:], wpack[:])
        nc.scalar.dma_start(bpack_sb[:], bpack[:])
        for b in range(2, B):
            nc.gpsimd.dma_start(
                xb_tiles[b][:].rearrange("d (h n) -> d h n", h=2),
                xTall[b].rearrange("h d n -> d h n"))
        # ema: bf16 in DRAM, cast to f32 during the SWDGE transfer
        nc.gpsimd.dma_start(
            ema_sb[:].rearrange("p (c n) -> p c n", c=4),
            ema.ap().rearrange("(c p) n -> p c n", p=128))
        nc.vector.memset(ones1_sb[:], 1.0)
        nc.vector.tensor_copy(onesc_sb[:], ones1_sb[:])  # f32r copy of ones

        # ---- engine warm-up: a full dummy chain on zeroed tiles --------
        # (no DRAM deps; keeps PE/ACT/DVE clocks up so batch 0's real
        # chain sees ~40ns semaphore wakeups instead of ~2us ones)
        dummy_x = const.tile([D, N], BF16)
        nc.vector.memset(dummy_x[:], 0.0)
        for w in range(2):
            psum_d = psum.tile([D, N], F32, tag="mm", bufs=4, name=f"psd{w}")
            nc.tensor.matmul(psum_d[:], dummy_x[:, 0:D], dummy_x[:],
                             start=True, stop=True)
            absd = absp.tile([D, N], BF16, tag="abs", bufs=4, name=f"absd{w}")
            nc.scalar.activation(absd[:], psum_d[:], AF.Abs)
            dume = work.tile([1, 1], F32, tag="se", bufs=6, name=f"dume{w}")
            dexp = work.tile([1, N], F32R, tag="ex", bufs=6, name=f"dexp{w}")
            nc.scalar.activation(dexp[:], psum_d[0:1, :], AF.Exp,
                                 scale=1.0, accum_out=dume[:])
            drs = work.tile([1, 1], F32, tag="rs", bufs=6, name=f"drs{w}")
            nc.vector.reciprocal(drs[:], dume[:])
            drr = work.tile([1, D], F32R, tag="rr", bufs=6, name=f"drr{w}")
            nc.vector.tensor_scalar_mul(drr[:], ones1_sb[:], drs[:])
            dcp = work.tile([128, N], F32, tag="repsb", bufs=B + 2,
                            name=f"dcp{w}")
            nc.vector.tensor_copy(dcp[:], psum_d[:])

        for b in range(B):
            xb = xb_tiles[b]
            psum_q = psum.tile([D, N], F32, tag="mm", bufs=4)
            nc.tensor.matmul(psum_q[:], wxq0, xb[:, 0:N],
                             start=True, stop=False)
            nc.tensor.matmul(psum_q[:], wxq1, xb[:, N:2 * N],
                             start=False, stop=True)
            absq = absp.tile([D, N], BF16, tag="abs", bufs=4)
            nc.scalar.activation(absq[:], psum_q[:], AF.Abs, bias=bq_l)
            psum_s = psum.tile([1, N], F32, tag="ps", bufs=2)
            nc.tensor.matmul(psum_s[:], wl20, xb[:, 0:N],
                             start=True, stop=False)
            nc.tensor.matmul(psum_s[:], wl21, xb[:, N:2 * N],
                             start=False, stop=False)
            psum_k = psum.tile([D, N], F32, tag="mm", bufs=4)
            nc.tensor.matmul(psum_k[:], wxk0, xb[:, 0:N],
                             start=True, stop=False)
            nc.tensor.matmul(psum_k[:], wxk1, xb[:, N:2 * N],
                             start=False, stop=True)
            absk = absp.tile([D, N], BF16, tag="abs", bufs=4)
            nc.scalar.activation(absk[:], psum_k[:], AF.Abs, bias=bk_l)
            nc.tensor.matmul(psum_s[:], aq_l, absq[:],
                             start=False, stop=False)
            nc.tensor.matmul(psum_s[:], ak_l, absk[:],
                             start=False, stop=True)

            # ---- softmax over free dim (no max-shift: |s| is O(1), exp
            # is safe in fp32 and softmax is shift-invariant) -------------
            expv = work.tile([1, N], F32R, tag="ex", bufs=6)
            sume = work.tile([1, 1], F32, tag="se", bufs=6)
            nc.scalar.activation(expv[:], psum_s[:], AF.Exp,
                                 scale=1.0, bias=c0_l, accum_out=sume[:])
            rsum = work.tile([1, 1], F32, tag="rs", bufs=6)
            nc.vector.reciprocal(rsum[:], sume[:])

            # pbar += PBAR_C * p  on a single [1,N] lane
            rsum_c = work.tile([1, 1], F32, tag="rc", bufs=6)
            nc.vector.tensor_scalar_mul(rsum_c[:], rsum[:], PBAR_C)
            if b == 0:
                nc.vector.tensor_scalar(pbar_sb[:], expv[:], rsum_c[:], None,
                                        op0=ALU.mult)
            else:
                nc.vector.scalar_tensor_tensor(
                    pbar_sb[:], expv[:], rsum_c[:], pbar_sb[:],
                    op0=ALU.mult, op1=ALU.add)

            # ---- alpha[b, i, :] = p_b for all i ------------------------
            # broadcast via PE: lhsT = rsum replicated (fp32r) so the
            # matmul computes rsum*expv = p on all 128 partitions.
            rsum_rep = work.tile([1, D], F32R, tag="rr", bufs=6)
            nc.vector.tensor_scalar_mul(rsum_rep[:], ones1_sb[:], rsum[:])
            psum_rep = psum.tile([128, N], F32, tag="rep", bufs=2)
            nc.tensor.matmul(psum_rep[:], rsum_rep[:], expv[:],
                             start=True, stop=True)
            rep_t = work.tile([128, N], F32, tag="repsb", bufs=B + 2)
            nc.vector.tensor_copy(rep_t[:], psum_rep[:])
            src = rep_t[:].rearrange(
                "p (o n) -> p o n", o=1).broadcast_to([128, 4, N])
            dst = alpha[b].rearrange("(p i) j -> p i j", p=128)
            if b % 2 == 0:
                nc.sync.dma_start(dst, src)
            else:
                nc.scalar.dma_start(dst, src)

        # ---- bias_log: broadcast pbar once, then 4 row-chunks ----------
        psum_pb = psum.tile([128, N], F32, tag="rep", bufs=2)
        nc.tensor.matmul(psum_pb[:], onesc_sb[:], pbar_sb[:],
                         start=True, stop=True)
        for c in range(4):
            u = work.tile([128, N], F32, tag="u", bufs=4)
            nc.vector.tensor_add(u[:], ema_sb[:, bass.ts(c, N)], psum_pb[:])
            v = work.tile([128, N], F32, tag="v", bufs=4)
            nc.vector.tensor_scalar_max(v[:], u[:], EPS / MOM)
            bias_t = work.tile([128, N], F32, tag="biassb", bufs=4)
            nc.scalar.activation(bias_t[:], v[:], AF.Ln, scale=MOM)
            src = bias_t[:].rearrange(
                "p (o n) -> p o n", o=1).broadcast_to([128, B // 2, N])
            dst = biaso.ap().rearrange("b (c p) j -> c p b j", c=4)[c]
            # two 1 MiB half-writes on alternating rings: halves the
            # straggler-engine tail of the final write and keeps both
            # HWDGE rings busy
            if c % 2 == 0:
                nc.sync.dma_start(dst[:, 0:B // 2], src)
                nc.scalar.dma_start(dst[:, B // 2:B], src)
            else:
                nc.scalar.dma_start(dst[:, 0:B // 2], src)
                nc.sync.dma_start(dst[:, B // 2:B], src)

    nc.compile()
    return nc


_NC_CACHE = None


def _get_nc():
    global _NC_CACHE
    if _NC_CACHE is None:
        _NC_CACHE = build()
    return _NC_CACHE


def make_in_maps(desc_embeddings, name_value_embeddings, W_fuse, b_fuse,
                 W_q, W_k, a, alpha_ema):
    """Host-side sharding / weight prep -> per-core input dicts."""
    bf16 = ml_dtypes.bfloat16
    desc = np.asarray(desc_embeddings, np.float32)
    nve = np.asarray(name_value_embeddings, np.float32)
    W_fuse = np.asarray(W_fuse, np.float32)
    b_fuse = np.asarray(b_fuse, np.float32)
    W_q = np.asarray(W_q, np.float32)
    W_k = np.asarray(W_k, np.float32)
    a = np.asarray(a, np.float32)
    alpha_ema = np.asarray(alpha_ema, np.float32)

    a_q = a[:, :R, 0]                      # [K,R]
    a_k = a[:, R:, 0]                      # [K,R]
    wlin = 0.3 * (np.einsum("kdr,kr->kd", W_q, a_q)
                  + np.einsum("kdr,kr->kd", W_k, a_k))  # [K,D]

    # fold the fusion layer into the per-basis projections
    Wxq = np.einsum("de,ker->kdr", W_fuse, W_q)   # [K,2D,R]
    Wxk = np.einsum("de,ker->kdr", W_fuse, W_k)
    bqv = np.einsum("e,ker->kr", b_fuse, W_q)     # [K,R]
    bkv = np.einsum("e,ker->kr", b_fuse, W_k)
    wl2v = np.einsum("de,ke->kd", W_fuse, wlin)   # [K,2D]
    c0v = wlin @ b_fuse                            # [K]

    # xTall[b] = [desc[b].T, nve[b].T] — shared across cores, bf16
    xTall = np.ascontiguousarray(
        np.stack([np.stack([desc[b].T, nve[b].T], axis=0)
                  for b in range(B)], axis=0)).astype(bf16)

    shared = dict(xTall=xTall)
    in_maps = []
    for m in range(N_CORES):
        wp = np.empty((D, WCOLS), np.float32)
        wp[:, 0 * R:1 * R] = Wxq[m][:D]
        wp[:, 1 * R:2 * R] = Wxq[m][D:]
        wp[:, 2 * R:3 * R] = Wxk[m][:D]
        wp[:, 3 * R:4 * R] = Wxk[m][D:]
        wp[:, 4 * R + 0] = wl2v[m][:D]
        wp[:, 4 * R + 1] = wl2v[m][D:]
        wp[:, 4 * R + 2] = 0.2 * a_q[m]
        wp[:, 4 * R + 3] = 0.2 * a_k[m]
        bp = np.empty((D, 3), np.float32)
        bp[:, 0] = bqv[m]
        bp[:, 1] = bkv[m]
        bp[:, 2] = c0v[m]
        in_maps.append(dict(
            shared,
            wpack=np.ascontiguousarray(wp).astype(bf16),
            bpack=np.ascontiguousarray(bp),
            ema=np.ascontiguousarray(alpha_ema[m]).astype(bf16)))
    return in_maps


def gather(results):
    alpha_full = np.stack([r["alpha"] for r in results], axis=1)
    bias_full = np.stack([r["bias"] for r in results], axis=1)
    return bias_full, alpha_full


def kernel(**inputs):
    nc = _get_nc()
    in_maps = make_in_maps(**inputs)
    res = run_bass_kernel_spmd(nc, in_maps, list(range(N_CORES)))
    return gather(res.results)
